# revision 2
# baseline (speedup 1.0000x reference)
"""DialogueRNN TP-8 Bass kernel for trn2 (8 NeuronCores, one chip).

Sharding: tensor-parallel over the hidden/gate dimension. Core i owns a
128-wide column slice of all three GRUs (g/p/e). Per step, one AllGather
exchanges the transposed state slices (g_k, qs_k, e_{k-1}).

Full-width state is kept transposed ([dim chunks on partitions, batch on
free], true-chunk-major — identical layout on every core, SPMD-clean).
The per-core party/select state additionally lives as untransposed local
slices [B, 128] so the p-GRU combine never needs a core-dependent chunk
index (masks become per-partition scalars there).
"""
import sys

sys.path.insert(0, "/opt/trn_rl_repo")
import numpy as np
import ml_dtypes

import concourse.bass as bass
import concourse.mybir as mybir
import concourse.tile as tile
from concourse import bacc
from concourse.bass_utils import run_bass_kernel_spmd
from concourse.masks import make_identity

F32 = mybir.dt.float32
BF16 = mybir.dt.bfloat16
AF = mybir.ActivationFunctionType
OP = mybir.AluOpType

T, B, S = 128, 64, 2
D = 1024          # all hidden dims
NC = 8            # cores
CH = 8            # k-chunks of 128 in D
SL = 128          # per-core slice width


def build_nc(t_steps=T, dbg=None):
    nc = bacc.Bacc("TRN2", target_bir_lowering=False, debug=False, num_devices=NC)

    # ---- external inputs (per-core values supplied via in_maps) ----
    xT = nc.dram_tensor("xT", [CH, 128, T * B], BF16, kind="ExternalInput")
    w_rz_g = nc.dram_tensor("w_rz_g", [128, 16, 2 * SL], F32, kind="ExternalInput")
    w_rz_p = nc.dram_tensor("w_rz_p", [128, 16, 2 * SL], F32, kind="ExternalInput")
    w_rz_e = nc.dram_tensor("w_rz_e", [128, 16, 2 * SL], F32, kind="ExternalInput")
    w_ni_g = nc.dram_tensor("w_ni_g", [128, CH, SL], F32, kind="ExternalInput")
    w_ni_p = nc.dram_tensor("w_ni_p", [128, CH, SL], F32, kind="ExternalInput")
    w_ni_e = nc.dram_tensor("w_ni_e", [128, CH, SL], F32, kind="ExternalInput")
    w_nh_g = nc.dram_tensor("w_nh_g", [128, CH, SL], F32, kind="ExternalInput")
    w_nh_p = nc.dram_tensor("w_nh_p", [128, CH, SL], F32, kind="ExternalInput")
    w_nh_e = nc.dram_tensor("w_nh_e", [128, CH, SL], F32, kind="ExternalInput")
    w_ho_g = nc.dram_tensor("w_ho_g", [128, CH, 3 * SL], BF16, kind="ExternalInput")
    w_ho_p = nc.dram_tensor("w_ho_p", [128, CH, 3 * SL], BF16, kind="ExternalInput")
    wattT_d = nc.dram_tensor("wattT", [128, CH], F32, kind="ExternalInput")
    maskrows = nc.dram_tensor("maskrows", [T + 1, 4 * B], F32, kind="ExternalInput")
    maskcols = nc.dram_tensor("maskcols", [T + 1, B, 4], F32, kind="ExternalInput")
    out_d = nc.dram_tensor("out", [t_steps, B, SL], F32, kind="ExternalOutput")

    with tile.TileContext(nc) as tc:
        const = tc.alloc_tile_pool(name="const", bufs=1)
        state = tc.alloc_tile_pool(name="state", bufs=1)
        work = tc.alloc_tile_pool(name="work", bufs=2)
        work3 = tc.alloc_tile_pool(name="work3", bufs=3)
        work1 = tc.alloc_tile_pool(name="work1", bufs=1)
        ps_gate = tc.alloc_tile_pool(name="ps_gate", bufs=1, space="PSUM")
        ps_misc = tc.alloc_tile_pool(name="ps_misc", bufs=4, space="PSUM")
        dram = tc.alloc_tile_pool(name="dram", bufs=2, space="DRAM")
        dram1 = tc.alloc_tile_pool(name="dram1", bufs=1, space="DRAM")

        # ---- resident weights ----
        Wrz = {}
        Wni = {}
        Wnh = {}
        for nm, drz, dni, dnh in (("g", w_rz_g, w_ni_g, w_nh_g),
                                  ("p", w_rz_p, w_ni_p, w_nh_p),
                                  ("e", w_rz_e, w_ni_e, w_nh_e)):
            Wrz[nm] = const.tile([128, 16, 2 * SL], F32, tag=f"wrz_{nm}", name=f"wrz_{nm}")
            nc.sync.dma_start(Wrz[nm][:], drz.ap())
            Wni[nm] = const.tile([128, CH, SL], F32, tag=f"wni_{nm}", name=f"wni_{nm}")
            nc.sync.dma_start(Wni[nm][:], dni.ap())
            Wnh[nm] = const.tile([128, CH, SL], F32, tag=f"wnh_{nm}", name=f"wnh_{nm}")
            nc.sync.dma_start(Wnh[nm][:], dnh.ap())
        Who = {}
        for nm, dho in (("g", w_ho_g), ("p", w_ho_p)):
            Who[nm] = const.tile([128, CH, 3 * SL], BF16, tag=f"who_{nm}", name=f"who_{nm}")
            nc.sync.dma_start(Who[nm][:], dho.ap())
        wattT = const.tile([128, CH], F32, tag="wattT", name="wattT")
        nc.sync.dma_start(wattT[:], wattT_d.ap())

        ident64 = const.tile([64, 64], F32, tag="id64", name="id64")
        make_identity(nc, ident64[:])
        ones_row = const.tile([1, 128], F32, tag="ones", name="ones")
        nc.vector.memset(ones_row[:], 1.0)

        # ---- persistent state ----
        def stT(tag):
            t_ = state.tile([128, CH, B], F32, tag=tag)
            nc.vector.memset(t_[:], 0.0)
            return t_

        gT = stT("gT")       # g_{k-1} full (transposed)
        qsT = stT("qsT")     # qs_{k-1} full
        eT = stT("eT")       # e_{k-2} full
        p0T = stT("p0T")
        p1T = stT("p1T")
        dT = stT("dT")
        preT = stT("preT")
        NattT = stT("NattT")
        Mrow = []
        for j in range(2):
            m_ = state.tile([1, B], F32, tag=f"Mrow{j}", name=f"Mrow{j}")
            nc.vector.memset(m_[:], 0.0)
            Mrow.append(m_)
        Zrow = state.tile([1, B], F32, tag="Zrow", name="Zrow")
        nc.vector.memset(Zrow[:], 1.0)

        def stL(tag):
            t_ = state.tile([B, SL], F32, tag=tag)
            nc.vector.memset(t_[:], 0.0)
            return t_

        p0L = stL("p0L")
        p1L = stL("p1L")
        dL = stL("dL")
        preL = stL("preL")
        g_loc_prev = stL("gloc_init")
        e_loc_prev = stL("eloc_init")
        qs_loc_prev = stL("qsloc_init")

        # ---- internal DRAM ----
        giU_g = dram1.tile([T, B, 3 * SL], F32, tag="giU_g", name="giU_g")
        giU_p = dram1.tile([T, B, 3 * SL], F32, tag="giU_p", name="giU_p")

        # ---- helpers ----
        def hoist_pair(pair):
            """Compute giU_g/p rows [2*pair, 2*pair+2) on PE."""
            xt = work3.tile([128, CH, 128], BF16, tag="hoist_x", name="hoist_x")
            nc.sync.dma_start(
                xt[:],
                xT.ap().rearrange("c p m -> p c m")[:, :, 2 * pair * B:(2 * pair + 2) * B],
            )
            for nm, giU in (("g", giU_g), ("p", giU_p)):
                ps = ps_misc.tile([128, 384], F32, tag="misc", name="misc")
                for j in range(CH):
                    nc.tensor.matmul(ps[:], xt[:, j, :], Who[nm][:, j, :],
                                     start=(j == 0), stop=(j == CH - 1),
                                     skip_group_check=True)
                hs = work3.tile([128, 3 * SL], F32, tag="hoist_s", name="hoist_s")
                nc.scalar.activation(hs[:], ps[:], AF.Copy)
                nc.sync.dma_start(
                    giU[2 * pair:2 * pair + 2].rearrange("t b n -> (t b) n"), hs[:]
                )

        def prefetch_giu(k):
            tg = work.tile([B, 3 * SL], F32, tag="giu_g_sb", name="giu_g_sb")
            nc.sync.dma_start(tg[:], giU_g[k])
            tp = work.tile([B, 3 * SL], F32, tag="giu_p_sb", name="giu_p_sb")
            nc.sync.dma_start(tp[:], giU_p[k])
            return tg, tp

        def prefetch_masks(k):
            stg = work.tile([1, 4 * B], F32, tag="mask_stage", name="mask_stage")
            nc.sync.dma_start(stg[:], maskrows.ap()[k:k + 1, :])
            stc = work.tile([B, 4], F32, tag="maskc_stage", name="maskc_stage")
            nc.sync.dma_start(stc[:], maskcols.ap()[k])
            return stg, stc

        def bcast_mask(stg):
            ps = ps_misc.tile([128, 384], F32, tag="misc", name="misc")
            nc.tensor.matmul(ps[:, 0:4 * B], ones_row[:], stg[:],
                             skip_group_check=True)
            mb = work.tile([128, 4, B], F32, tag="maskB", name="maskB")
            nc.scalar.activation(mb[:].rearrange("p c b -> p (c b)"), ps[:, 0:4 * B],
                                 AF.Copy)
            return mb

        def bc3(ap2d):
            """[128, B] slice -> broadcast to [128, CH, B]."""
            return ap2d[:, None, :].to_broadcast([128, CH, B])

        def combine(nm, ps, giu, h_old_ap, out_tag):
            """GRU pointwise. ps = [rz(256) | ni(128) | nh(128)] psum tile.
            giu: SBUF [B, 384] hoisted input-side pre-acts (None for e-GRU).
            Returns new local slice [B, SL]."""
            if giu is not None:
                prerz = work1.tile([B, 2 * SL], F32, tag=f"prerz_{nm}", name=f"prerz_{nm}")
                nc.vector.tensor_add(prerz[:], ps[:, 0:2 * SL], giu[:, 0:2 * SL])
                sig_src = prerz[:]
            else:
                sig_src = ps[:, 0:2 * SL]
            sig = work1.tile([B, 2 * SL], F32, tag=f"sig_{nm}", name=f"sig_{nm}")
            nc.scalar.activation(sig[:], sig_src, AF.Sigmoid)
            c1 = work1.tile([B, SL], F32, tag=f"c1_{nm}", name=f"c1_{nm}")
            nc.vector.tensor_mul(c1[:], sig[:, 0:SL], ps[:, 3 * SL:4 * SL])
            c3 = work1.tile([B, SL], F32, tag=f"c3_{nm}", name=f"c3_{nm}")
            if giu is not None:
                c2 = work1.tile([B, SL], F32, tag=f"c2_{nm}", name=f"c2_{nm}")
                nc.vector.tensor_add(c2[:], ps[:, 2 * SL:3 * SL], giu[:, 2 * SL:3 * SL])
                nc.vector.tensor_add(c3[:], c1[:], c2[:])
            else:
                nc.vector.tensor_add(c3[:], c1[:], ps[:, 2 * SL:3 * SL])
            n_ = work1.tile([B, SL], F32, tag=f"n_{nm}", name=f"n_{nm}")
            nc.scalar.activation(n_[:], c3[:], AF.Tanh)
            w_ = work1.tile([B, SL], F32, tag=f"w_{nm}", name=f"w_{nm}")
            nc.vector.tensor_sub(w_[:], h_old_ap, n_[:])
            a_ = work1.tile([B, SL], F32, tag=f"a_{nm}", name=f"a_{nm}")
            nc.vector.tensor_mul(a_[:], sig[:, SL:2 * SL], w_[:])
            h_ = work.tile([B, SL], F32, tag=out_tag)
            nc.vector.tensor_add(h_[:], n_[:], a_[:])
            return h_

        # ---- prologue ----
        n_pairs = (t_steps + 1) // 2
        for pair in range(min(3, n_pairs)):
            hoist_pair(pair)
        giu_next = prefetch_giu(0)
        masks_next = prefetch_masks(0)
        maskB_next = bcast_mask(masks_next[0])

        bounce_out_prev = None

        for k in range(t_steps):
            giu_g_sb, giu_p_sb = giu_next
            maskB = maskB_next
            mcol = masks_next[1]

            # 1. land AG_{k-1} results
            if k > 0:
                bo = bounce_out_prev
                for s_idx, st_tile in ((0, gT), (1, qsT), (2, eT)):
                    src = bo[:][:, :, s_idx, :].rearrange("c p b -> p c b")
                    nc.sync.dma_start(st_tile[:], src)

            # 2. attention fold of g_{k-1} -> cT
            if k > 0:
                s_ps = ps_misc.tile([128, 384], F32, tag="misc", name="misc")
                for j in range(CH):
                    nc.tensor.matmul(s_ps[0:1, 0:B], wattT[:, j:j + 1], gT[:, j, :],
                                     start=(j == 0), stop=(j == CH - 1),
                                     skip_group_check=True)
                if k == 1:
                    nc.vector.tensor_copy(NattT[:], gT[:])
                    nc.vector.tensor_copy(Mrow[1][:], s_ps[0:1, 0:B])
                    cT_ap = gT[:]
                else:
                    mprev, mcur = Mrow[(k - 1) % 2], Mrow[k % 2]
                    nc.vector.tensor_max(mcur[:], mprev[:], s_ps[0:1, 0:B])
                    exin = work1.tile([1, 2 * B], F32, tag="exin", name="exin")
                    nc.vector.tensor_sub(exin[:, 0:B], mprev[:], mcur[:])
                    nc.vector.tensor_sub(exin[:, B:2 * B], s_ps[0:1, 0:B], mcur[:])
                    sg = work1.tile([1, 2 * B], F32, tag="sg", name="sg")
                    nc.scalar.activation(sg[:], exin[:], AF.Sigmoid)
                    om = work1.tile([1, 2 * B], F32, tag="om", name="om")
                    nc.vector.tensor_scalar(om[:], sg[:], -1.0, 1.0, OP.mult, OP.add)
                    rc = work1.tile([1, 2 * B], F32, tag="rc", name="rc")
                    nc.vector.reciprocal(rc[:], om[:])
                    ex = work1.tile([1, 3 * B], F32, tag="ex", name="ex")
                    nc.vector.tensor_mul(ex[:, 0:2 * B], sg[:], rc[:])
                    t1 = work1.tile([1, B], F32, tag="t1", name="t1")
                    nc.vector.tensor_mul(t1[:], Zrow[:], ex[:, 0:B])
                    nc.vector.tensor_add(Zrow[:], t1[:], ex[:, B:2 * B])
                    nc.vector.reciprocal(ex[:, 2 * B:3 * B], Zrow[:])
                    ps_bc = ps_misc.tile([128, 384], F32, tag="misc", name="misc")
                    nc.tensor.matmul(ps_bc[:, 0:3 * B], ones_row[:], ex[:],
                                     skip_group_check=True)
                    f2 = work1.tile([128, CH, B], F32, tag="f2", name="f2")
                    nc.vector.tensor_mul(f2[:], NattT[:], bc3(ps_bc[:, 0:B]))
                    f3 = work1.tile([128, CH, B], F32, tag="f3", name="f3")
                    nc.vector.tensor_mul(f3[:], gT[:], bc3(ps_bc[:, B:2 * B]))
                    nc.vector.tensor_add(NattT[:], f2[:], f3[:])
                    cT = work1.tile([128, CH, B], F32, tag="cT", name="cT")
                    nc.vector.tensor_mul(cT[:], NattT[:], bc3(ps_bc[:, 2 * B:3 * B]))
                    cT_ap = cT[:]

                # 3. select q0 (transposed, full) on DVE
                s1 = work1.tile([128, CH, B], F32, tag="s1", name="s1")
                nc.vector.tensor_sub(s1[:], qsT[:], preT[:])
                s2 = work1.tile([128, CH, B], F32, tag="s2", name="s2")
                nc.vector.tensor_mul(s2[:], s1[:],
                                     maskB[:, 3:4, :].to_broadcast([128, CH, B]))
                q0T = work1.tile([128, CH, B], F32, tag="q0T", name="q0T")
                nc.vector.tensor_add(q0T[:], preT[:], s2[:])
                q0T_ap = q0T[:]

                # 3b. local (untransposed) select on gpsimd -> q0 slice
                l1 = work1.tile([B, SL], F32, tag="l1", name="l1")
                nc.gpsimd.tensor_sub(l1[:], qs_loc_prev[:], preL[:])
                nc.gpsimd.tensor_scalar_mul(l1[:], l1[:], mcol[:, 3:4])
                q0L = work1.tile([B, SL], F32, tag="q0L", name="q0L")
                nc.gpsimd.tensor_add(q0L[:], preL[:], l1[:])
                q0L_ap = q0L[:]

            # 4+5. gate GEMMs
            ps_g = ps_gate.tile([B, 4 * SL], F32, tag="ps_g", name="ps_g")
            ps_p = ps_gate.tile([B, 4 * SL], F32, tag="ps_p", name="ps_p")
            ps_e = ps_gate.tile([B, 4 * SL], F32, tag="ps_e", name="ps_e")
            if k > 0:
                # AG-dependent (ready first): s-row done above; e-GRU fills PE
                # while DVE computes q0T / cT.
                for j in range(CH):
                    nc.tensor.matmul(ps_e[:, 0:2 * SL], qsT[:, j, :], Wrz["e"][:, j, :],
                                     start=(j == 0), stop=False, skip_group_check=True)
                for j in range(CH):
                    nc.tensor.matmul(ps_e[:, 0:2 * SL], eT[:, j, :], Wrz["e"][:, CH + j, :],
                                     start=False, stop=(j == CH - 1),
                                     skip_group_check=True)
                for j in range(CH):
                    nc.tensor.matmul(ps_e[:, 2 * SL:3 * SL], qsT[:, j, :], Wni["e"][:, j, :],
                                     start=(j == 0), stop=(j == CH - 1),
                                     skip_group_check=True)
                for j in range(CH):
                    nc.tensor.matmul(ps_e[:, 3 * SL:4 * SL], eT[:, j, :], Wnh["e"][:, j, :],
                                     start=(j == 0), stop=(j == CH - 1),
                                     skip_group_check=True)
                # g-GRU: rz contiguous [gT-side; q0-side]
                for j in range(CH):
                    nc.tensor.matmul(ps_g[:, 0:2 * SL], gT[:, j, :], Wrz["g"][:, CH + j, :],
                                     start=(j == 0), stop=False, skip_group_check=True)
                for j in range(CH):
                    nc.tensor.matmul(ps_g[:, 0:2 * SL], q0T_ap[:, j, :], Wrz["g"][:, j, :],
                                     start=False, stop=(j == CH - 1),
                                     skip_group_check=True)
                for j in range(CH):
                    nc.tensor.matmul(ps_g[:, 3 * SL:4 * SL], gT[:, j, :], Wnh["g"][:, j, :],
                                     start=(j == 0), stop=(j == CH - 1),
                                     skip_group_check=True)
                for j in range(CH):
                    nc.tensor.matmul(ps_g[:, 2 * SL:3 * SL], q0T_ap[:, j, :], Wni["g"][:, j, :],
                                     start=(j == 0), stop=(j == CH - 1),
                                     skip_group_check=True)
                # p-GRU: nh (q0) first, rz contiguous [q0-side; cT-side], ni (cT)
                for j in range(CH):
                    nc.tensor.matmul(ps_p[:, 3 * SL:4 * SL], q0T_ap[:, j, :], Wnh["p"][:, j, :],
                                     start=(j == 0), stop=(j == CH - 1),
                                     skip_group_check=True)
                for j in range(CH):
                    nc.tensor.matmul(ps_p[:, 0:2 * SL], q0T_ap[:, j, :], Wrz["p"][:, CH + j, :],
                                     start=(j == 0), stop=False, skip_group_check=True)
                for j in range(CH):
                    nc.tensor.matmul(ps_p[:, 0:2 * SL], cT_ap[:, j, :], Wrz["p"][:, j, :],
                                     start=False, stop=(j == CH - 1),
                                     skip_group_check=True)
                for j in range(CH):
                    nc.tensor.matmul(ps_p[:, 2 * SL:3 * SL], cT_ap[:, j, :], Wni["p"][:, j, :],
                                     start=(j == 0), stop=(j == CH - 1),
                                     skip_group_check=True)
            else:
                # k == 0: all contraction inputs are zero and biases are zero.
                nc.vector.memset(ps_g[:], 0.0)
                nc.vector.memset(ps_p[:], 0.0)
                nc.vector.memset(ps_e[:], 0.0)
                q0L_ap = stL("q0L_zero")[:]

            # 6-11. combines, transposes, bounce writes
            g_loc = combine("g", ps_g, giu_g_sb, g_loc_prev[:], "g_loc")
            qs_loc = combine("p", ps_p, giu_p_sb, q0L_ap, "qs_loc")
            e_loc = combine("e", ps_e, None, e_loc_prev[:], "e_loc")

            stage3 = work.tile([128, 3, B], F32, tag="stage3", name="stage3")
            for s_idx, loc in ((0, g_loc), (1, qs_loc), (2, e_loc)):
                pst = ps_misc.tile([128, 384], F32, tag="misc", name="misc")
                nc.tensor.transpose(pst[:, 0:B], loc[:], ident64[:])
                nc.scalar.activation(stage3[:, s_idx, :], pst[:, 0:B], AF.Copy)

            if k > 0:
                if dbg == "gT":
                    nc.sync.dma_start(
                        out_d.ap()[k - 1],
                        bounce_out_prev[:][0, :, 0, :].rearrange("p b -> b p"))
                elif dbg == "cT1":
                    nc.sync.dma_start(
                        out_d.ap()[k - 1],
                        bounce_out_prev[:][1, :, 0, :].rearrange("p b -> b p"))
                elif dbg == "q0T":
                    scr = dram.tile([128, B], F32, tag="scr", name="scr")
                    nc.sync.dma_start(scr[:], q0T_ap[:, 0, :])
                    nc.sync.dma_start(out_d.ap()[k - 1],
                                      scr[:].rearrange("p b -> b p"))
                elif dbg == "eqB":
                    scr = dram.tile([128, B], F32, tag="scr", name="scr")
                    nc.sync.dma_start(scr[:], maskB[:, 3, :])
                    nc.sync.dma_start(out_d.ap()[k - 1],
                                      scr[:].rearrange("p b -> b p"))
                elif dbg in ("psg", "psg_h"):
                    pd = work1.tile([B, SL], F32, tag="pd", name="pd")
                    nc.scalar.activation(pd[:], ps_g[:, 0:SL], AF.Copy)
                    nc.sync.dma_start(out_d.ap()[k - 1], pd[:])
                else:
                    dbg_src = {None: e_loc, "qs": qs_loc, "g": g_loc,
                               "q0": q0L}.get(dbg, e_loc)
                    nc.sync.dma_start(out_d.ap()[k - 1], dbg_src[:])

            bounce_in = dram.tile([128, 3, B], F32, tag="bounce_in", name="bounce_in")
            nc.sync.dma_start(bounce_in[:], stage3[:])
            bounce_out = dram.tile([NC, 128, 3, B], F32, tag="bounce_out", name="bounce_out")
            nc.gpsimd.collective_compute(
                "AllGather",
                OP.bypass,
                ins=[bounce_in[:].rearrange("p s b -> (p s) b")],
                outs=[bounce_out[:].rearrange("c p s b -> (c p s) b")],
                replica_groups=[list(range(NC))],
            )
            bounce_out_prev = bounce_out

            # 13. merge qs_{k-1} into parties, then preselect for k+1
            if k > 0:
                a0B = maskB[:, 0:1, :].to_broadcast([128, CH, B])
                a1B = maskB[:, 1:2, :].to_broadcast([128, CH, B])
                mg1 = work1.tile([128, CH, B], F32, tag="mg1", name="mg1")
                nc.gpsimd.tensor_sub(mg1[:], qsT[:], p0T[:])
                nc.gpsimd.tensor_mul(mg1[:], mg1[:], a0B)
                nc.gpsimd.tensor_add(p0T[:], p0T[:], mg1[:])
                mg2 = work1.tile([128, CH, B], F32, tag="mg2", name="mg2")
                nc.gpsimd.tensor_sub(mg2[:], qsT[:], p1T[:])
                nc.gpsimd.tensor_mul(mg2[:], mg2[:], a1B)
                nc.gpsimd.tensor_add(p1T[:], p1T[:], mg2[:])
                nc.gpsimd.tensor_sub(dT[:], p1T[:], p0T[:])
                # local mirror
                lm = work1.tile([B, SL], F32, tag="lm", name="lm")
                nc.gpsimd.tensor_sub(lm[:], qs_loc_prev[:], p0L[:])
                nc.gpsimd.tensor_scalar_mul(lm[:], lm[:], mcol[:, 0:1])
                nc.gpsimd.tensor_add(p0L[:], p0L[:], lm[:])
                nc.gpsimd.tensor_sub(lm[:], qs_loc_prev[:], p1L[:])
                nc.gpsimd.tensor_scalar_mul(lm[:], lm[:], mcol[:, 1:2])
                nc.gpsimd.tensor_add(p1L[:], p1L[:], lm[:])
                nc.gpsimd.tensor_sub(dL[:], p1L[:], p0L[:])
            a1nB = maskB[:, 2:3, :].to_broadcast([128, CH, B])
            mg3 = work1.tile([128, CH, B], F32, tag="mg3", name="mg3")
            nc.gpsimd.tensor_mul(mg3[:], dT[:], a1nB)
            nc.gpsimd.tensor_add(preT[:], p0T[:], mg3[:])
            lm3 = work1.tile([B, SL], F32, tag="lm3", name="lm3")
            nc.gpsimd.tensor_scalar_mul(lm3[:], dL[:], mcol[:, 2:3])
            nc.gpsimd.tensor_add(preL[:], p0L[:], lm3[:])

            # 14-16. next-step staging: masks, hoist, giU prefetch
            masks_next = prefetch_masks(k + 1)
            maskB_next = bcast_mask(masks_next[0])
            pair = k // 2 + 3
            if k % 2 == 0 and pair < n_pairs:
                hoist_pair(pair)
            if k + 1 < t_steps:
                giu_next = prefetch_giu(k + 1)

            g_loc_prev = g_loc
            e_loc_prev = e_loc
            qs_loc_prev = qs_loc

        # ---- epilogue: e_{T-1} = GRU(qs_{T-1}, e_{T-2}) ----
        bo = bounce_out_prev
        for s_idx, st_tile in ((1, qsT), (2, eT)):
            src = bo[:][:, :, s_idx, :].rearrange("c p b -> p c b")
            nc.sync.dma_start(st_tile[:], src)
        ps_e = ps_gate.tile([B, 4 * SL], F32, tag="ps_e", name="ps_e")
        for j in range(CH):
            nc.tensor.matmul(ps_e[:, 0:2 * SL], qsT[:, j, :], Wrz["e"][:, j, :],
                             start=(j == 0), stop=False, skip_group_check=True)
        for j in range(CH):
            nc.tensor.matmul(ps_e[:, 0:2 * SL], eT[:, j, :], Wrz["e"][:, CH + j, :],
                             start=False, stop=(j == CH - 1), skip_group_check=True)
        for j in range(CH):
            nc.tensor.matmul(ps_e[:, 2 * SL:3 * SL], qsT[:, j, :], Wni["e"][:, j, :],
                             start=(j == 0), stop=(j == CH - 1), skip_group_check=True)
        for j in range(CH):
            nc.tensor.matmul(ps_e[:, 3 * SL:4 * SL], eT[:, j, :], Wnh["e"][:, j, :],
                             start=(j == 0), stop=(j == CH - 1), skip_group_check=True)
        e_loc = combine("e", ps_e, None, e_loc_prev[:], "e_loc")
        nc.sync.dma_start(out_d.ap()[t_steps - 1], e_loc[:])

        for p in (dram1, dram, ps_misc, ps_gate, work1, work3, work, state, const):
            p.release()

    nc.compile()
    return nc


def host_prep(inputs, t_steps=T):
    f32 = np.float32
    feats = np.asarray(inputs["features"], f32)
    spk = np.asarray(inputs["speakers"], f32)
    Wg_ih = np.asarray(inputs["Wih_g"], f32); Wg_hh = np.asarray(inputs["Whh_g"], f32)
    Wp_ih = np.asarray(inputs["Wih_p"], f32); Wp_hh = np.asarray(inputs["Whh_p"], f32)
    We_ih = np.asarray(inputs["Wih_e"], f32); We_hh = np.asarray(inputs["Whh_e"], f32)
    watt = np.asarray(inputs["w_att"], f32)
    for bn in ("bih_g", "bhh_g", "bih_p", "bhh_p", "bih_e", "bhh_e"):
        assert not np.any(np.asarray(inputs[bn])), f"nonzero bias {bn} unsupported"

    # xT: (CH, 128, T*B), T-major columns
    xT = np.ascontiguousarray(feats.reshape(T * B, D).T.reshape(CH, 128, T * B)).astype(ml_dtypes.bfloat16)

    # mask rows: row k = [a0_{k-1} | a1_{k-1} | a1_{k+1} | eq_k]
    a0 = spk[:, :, 0]; a1 = spk[:, :, 1]
    mrows = np.zeros((T + 1, 4, B), f32)
    for k in range(T + 1):
        if 1 <= k <= T:
            mrows[k, 0] = a0[k - 1]; mrows[k, 1] = a1[k - 1]
        if k + 1 < T:
            mrows[k, 2] = a1[k + 1]
        if 1 <= k < T:
            mrows[k, 3] = a0[k] * a0[k - 1] + a1[k] * a1[k - 1]
    mcols = np.ascontiguousarray(mrows.transpose(0, 2, 1))   # (T+1, B, 4)
    mrows = mrows.reshape(T + 1, 4 * B)

    def chunked_rhs(wmat):
        """(K, N) -> (128, K/128, N) chunk-major rhs layout."""
        Kd, Nd = wmat.shape
        return np.ascontiguousarray(wmat.reshape(Kd // 128, 128, Nd).transpose(1, 0, 2))

    wattT_full = np.ascontiguousarray(watt.reshape(CH, 128).T)  # (128, 8)

    in_maps = []
    for i in range(NC):
        rows = slice(i * SL, (i + 1) * SL)

        def rz_weights(Wx, Whh):
            r_x = Wx[rows]; z_x = Wx[D + i * SL: D + (i + 1) * SL]
            r_h = Whh[rows]; z_h = Whh[D + i * SL: D + (i + 1) * SL]
            xs = np.concatenate([r_x, z_x], 0).T       # (1024, 256)
            hs = np.concatenate([r_h, z_h], 0).T
            return np.concatenate([chunked_rhs(xs), chunked_rhs(hs)], 1)  # (128,16,256)

        def n_weights(Wmat):
            n_w = Wmat[2 * D + i * SL: 2 * D + (i + 1) * SL].T   # (1024, 128)
            return chunked_rhs(n_w)

        Wg_x = Wg_ih[:, D:]; Wg_U = Wg_ih[:, :D]
        Wp_x = Wp_ih[:, D:]; Wp_U = Wp_ih[:, :D]

        def ho_weights(WU):
            wsl = np.concatenate([WU[rows], WU[D + i * SL: D + (i + 1) * SL],
                                  WU[2 * D + i * SL: 2 * D + (i + 1) * SL]], 0).T
            return chunked_rhs(wsl).astype(ml_dtypes.bfloat16)   # (128, 8, 384)

        in_maps.append({
            "xT": xT,
            "w_rz_g": rz_weights(Wg_x, Wg_hh),
            "w_rz_p": rz_weights(Wp_x, Wp_hh),
            "w_rz_e": rz_weights(We_ih, We_hh),
            "w_ni_g": n_weights(Wg_x),
            "w_ni_p": n_weights(Wp_x),
            "w_ni_e": n_weights(We_ih),
            "w_nh_g": n_weights(Wg_hh),
            "w_nh_p": n_weights(Wp_hh),
            "w_nh_e": n_weights(We_hh),
            "w_ho_g": ho_weights(Wg_U),
            "w_ho_p": ho_weights(Wp_U),
            "wattT": wattT_full,
            "maskrows": mrows,
            "maskcols": mcols,
        })
    return in_maps


_NC_CACHE = {}


def run(inputs, t_steps=T, dbg=None):
    key = (t_steps, dbg)
    if key not in _NC_CACHE:
        _NC_CACHE[key] = build_nc(t_steps, dbg)
    nc = _NC_CACHE[key]
    in_maps = host_prep(inputs, t_steps)
    res = run_bass_kernel_spmd(nc, in_maps, list(range(NC)))
    out = np.concatenate([res.results[c]["out"] for c in range(NC)], axis=2)
    return out




def kernel(**inputs) -> np.ndarray:
    """Harness entry: full inputs in, full (T, B, 1024) float32 output."""
    out = run(inputs, T)
    return np.ascontiguousarray(out.astype(np.float32))


# ---- timing helper for test.py (not used by the grading call) ----

def measure_exec_ns(inputs, n_calls=8):
    """Min wall of repeated NEFF executions with device-resident inputs.

    Upper-bounds device execution time: includes PJRT dispatch but excludes
    host->device transfer of the (large) inputs and NEFF compilation.
    """
    import time
    import jax
    from jax.sharding import Mesh, PartitionSpec
    from jax.experimental.shard_map import shard_map
    from concourse import bass2jax
    from concourse.bass2jax import _bass_exec_p, partition_id_tensor

    if (T, None) not in _NC_CACHE:
        _NC_CACHE[(T, None)] = build_nc(T, None)
    nc = _NC_CACHE[(T, None)]
    bass2jax.install_neuronx_cc_hook()
    pname = nc.partition_id_tensor.name if nc.partition_id_tensor else None
    in_names, out_names, out_avals, zero_outs = [], [], [], []
    for alloc in nc.m.functions[0].allocations:
        if not isinstance(alloc, mybir.MemoryLocationSet):
            continue
        name = alloc.memorylocations[0].name
        if alloc.kind == "ExternalInput":
            if name != pname:
                in_names.append(name)
        elif alloc.kind == "ExternalOutput":
            shape = tuple(alloc.tensor_shape)
            dtype = mybir.dt.np(alloc.dtype)
            out_names.append(name)
            out_avals.append(jax.core.ShapedArray(shape, dtype))
            zero_outs.append(np.zeros(shape, dtype))
    n_params, n_outs = len(in_names), len(out_avals)
    all_in = list(in_names) + list(out_names)
    if pname is not None:
        all_in.append(pname)

    def _body(*args):
        operands = list(args)
        if pname is not None:
            operands.append(partition_id_tensor())
        outs = _bass_exec_p.bind(
            *operands, out_avals=tuple(out_avals), in_names=tuple(all_in),
            out_names=tuple(out_names), lowering_input_output_aliases=(),
            sim_require_finite=True, sim_require_nnan=True, nc=nc)
        return tuple(outs)

    devices = jax.devices()[:NC]
    mesh = Mesh(np.asarray(devices), ("core",))
    sharded = jax.jit(
        shard_map(_body, mesh=mesh,
                  in_specs=(PartitionSpec("core"),) * (n_params + n_outs),
                  out_specs=(PartitionSpec("core"),) * n_outs,
                  check_rep=False),
        keep_unused=True)

    im = host_prep(inputs, T)
    din = [jax.device_put(np.concatenate([im[c][nm] for c in range(NC)], axis=0))
           for nm in in_names]
    din += [jax.device_put(np.zeros((NC * z.shape[0], *z.shape[1:]), z.dtype))
            for z in zero_outs]
    for a in din:
        a.block_until_ready()
    times = []
    for _ in range(n_calls):
        t0 = time.time()
        outs = sharded(*din)
        for o in outs:
            o.block_until_ready()
        times.append(time.time() - t0)
    out = np.asarray(outs[0]).reshape(NC, T, B, SL)
    full = np.concatenate([out[c] for c in range(NC)], axis=2)
    return min(times) * 1e9, full
